# revision 11
# baseline (speedup 1.0000x reference)
"""EvolveGCN (2-layer) Trainium2 Bass kernel, 8-way sharded. v2.

Algebraic reduction (same as v1): only h2[T-1] is returned and the mat-GRU
weight evolution is data-independent, so the whole model collapses to
    W1* = matGRU^4(W1);  W2* = matGRU^4(W2)      (host, fp64)
    h1  = rrelu(A3 @ (X3 @ W1*));  out = rrelu(A3 @ (h1 @ W2*))

v2 device scheme (per core, nodes row-partitioned):
- Node relabeling pi per core: 196 windows x 32 nodes, LPT-balanced by degree
  so every window receives ~510 edges -> the shared chunk schedule is
  ceil(max_core/128) = 4 chunks everywhere (~0.4% slot padding vs 16% in v1).
- Table (X@W1*, h1@W2*) is fp16 [50176, 128], built on device from bf16
  inputs, AllGathered (SIM1: emulated by 8 DMA copies). The gather views it
  as [25088, 256] super-rows: one 512B descriptor per edge (same modeled DMA
  cost as v1's 256B descriptor, but int16 super-row indices kill the A/B
  index-range split and its per-window double ceil).
- Messages: SWDGE dma_gather per segment (8 windows, ~32 chunks).
- Scatter: per chunk two one-hot fp16 matmuls (lo/hi half of each 512B slot)
  accumulating into a [32,128] PSUM window; S streamed once from DRAM
  (12.9MB) and kept resident for layer 2.
- rrelu emit split across ACT (x*SLOPE) and DVE (max) engines; h1 kept in
  SBUF as bf16; layer-2 table build (transpose + matmul) interleaved into
  the layer-1 segment loop. Output unscaling (2^-k2) done on host.
"""

import sys

for _p in ("/opt/trn_rl_repo",):
    if _p not in sys.path:
        sys.path.insert(0, _p)

import heapq

import ml_dtypes
import numpy as np

T, N, E, F = 4, 50000, 800000, 128
NC = 8
NPC = N // NC            # 6250 nodes per core
W = 32                   # window rows
NWIN = 196               # windows per core
RTP = NWIN * W           # 6272 padded rows per core
NT = RTP // 128          # 49 row tiles per core
SEG_WINS = 8             # windows per gather segment (2 row tiles)
SLOPE = 11.0 / 48.0      # torch RReLU eval negative slope

SIM1 = False             # single-core, no-collective variant for TimelineSim

BF16 = ml_dtypes.bfloat16


def _evolve(W0, gW, gU, gb, steps=T):
    def sig(x):
        return 1.0 / (1.0 + np.exp(-x))

    Q = W0.astype(np.float64)
    gW = gW.astype(np.float64)
    gU = gU.astype(np.float64)
    gb = gb.astype(np.float64)
    for _ in range(steps):
        z = sig(gW[0] @ Q + gU[0] @ Q + gb[0])
        r = sig(gW[1] @ Q + gU[1] @ Q + gb[1])
        h = np.tanh(gW[2] @ Q + gU[2] @ (r * Q) + gb[2])
        Q = (1.0 - z) * Q + z * h
    return Q.astype(np.float32)


def _lpt_windows(deg):
    """Assign all N nodes (by degree) to NC*NWIN global windows of W slots,
    balancing per-window degree sums. Nodes may land on any core — this
    balances core totals and window sums at once. Returns pos_g[node] in
    [0, NC*RTP)."""
    nbins = NC * NWIN
    order = np.argsort(-deg, kind="stable")
    pos_g = np.empty(N, np.int64)
    cnt = np.zeros(nbins, np.int32)
    heap = [(0.0, w) for w in range(nbins)]
    heapq.heapify(heap)
    for node in order:
        while True:
            s, w = heapq.heappop(heap)
            if cnt[w] < W:
                break
        pos_g[node] = w * W + cnt[w]
        cnt[w] += 1
        if cnt[w] < W:
            heapq.heappush(heap, (s + deg[node], w))
    return pos_g


def _rrelu(x):
    return np.where(x >= 0, x, SLOPE * x)


def _host_prep(features, adj_row, adj_col, adj_val, W1, g1_W, g1_U, g1_b,
               W2, g2_W, g2_U, g2_b):
    X = np.asarray(features[T - 1], dtype=np.float32)
    row = np.asarray(adj_row[T - 1], dtype=np.int64)
    col = np.asarray(adj_col[T - 1], dtype=np.int64)
    val = np.asarray(adj_val[T - 1], dtype=np.float32)

    W1f = _evolve(np.asarray(W1), np.asarray(g1_W), np.asarray(g1_U), np.asarray(g1_b))
    W2f = _evolve(np.asarray(W2), np.asarray(g2_W), np.asarray(g2_U), np.asarray(g2_b))

    # --- node relabeling: global LPT window balancing by (row-)degree;
    # a node's core is whichever window it lands in
    deg = np.bincount(row, minlength=N).astype(np.float64)
    newpos_g = _lpt_windows(deg)                                  # node -> table row

    trow_g = newpos_g[row]
    tcol_g = newpos_g[col]
    ecore = trow_g // RTP
    trl = trow_g % RTP
    ewin = trl // W
    erow = trl % W
    esup = tcol_g // 2
    epar = tcol_g % 2

    # --- shared chunk schedule
    counts = np.zeros((NC, NWIN), np.int64)
    np.add.at(counts, (ecore, ewin), 1)
    CC = np.maximum(1, -(-counts.max(axis=0) // 128))   # chunks per window
    base = np.zeros(NWIN + 1, np.int64)
    base[1:] = np.cumsum(CC)
    NCH = int(base[-1])
    NSLOT = NCH * 128

    segs = []
    for w0 in range(0, NWIN, SEG_WINS):
        w1 = min(w0 + SEG_WINS, NWIN)
        segs.append((w0, w1, int(base[w0]), int(base[w1])))
    SEGCH = max(c1 - c0 for _, _, c0, c1 in segs)

    # --- per-core slot data
    idx = np.zeros((NC, 128, NSLOT // 16), np.int16)
    sv = np.zeros((NC, 128, NCH * 2 * W), np.float16)
    for i in range(NC):
        m = ecore == i
        w_, r_, s_, p_, v_ = ewin[m], erow[m], esup[m], epar[m], val[m]
        o = np.argsort(w_, kind="stable")
        w_, r_, s_, p_, v_ = w_[o], r_[o], s_[o], p_[o], v_[o]
        winstart = np.searchsorted(w_, np.arange(NWIN))
        pos = np.arange(w_.size) - winstart[w_]
        assert (pos < CC[w_] * 128).all()
        slot = base[w_] * 128 + pos
        flat = np.zeros(NSLOT, np.int16)
        flat[slot] = s_.astype(np.int16)
        wrap = flat.reshape(-1, 16).T
        idx[i] = np.tile(wrap, (8, 1))
        c_ = slot // 128
        pp_ = slot % 128
        sv[i][pp_, c_ * 2 * W + p_ * W + r_] = v_.astype(np.float16)

    # --- permuted, transposed, bf16 features
    ncore = newpos_g // RTP
    nlocal = newpos_g % RTP
    xs = np.zeros((NC, 128, RTP), BF16)
    for i in range(NC):
        m = ncore == i
        Xp = np.zeros((RTP, F), np.float32)
        Xp[nlocal[m]] = X[m]
        xs[i] = Xp.T.astype(BF16)

    # --- weight folding + pow2 scale calibration (keeps fp16 tables in a
    # comfortable range; inverse applied to the output on host)
    XW = X.astype(BF16).astype(np.float32) @ W1f
    k1 = int(np.floor(np.log2(16.0 / np.abs(XW).max())))
    try:
        from scipy.sparse import csr_matrix

        A = csr_matrix((val, (row, col)), shape=(N, N))
        pre1 = A @ XW
    except Exception:
        pre1 = np.zeros((N, F), np.float32)
        np.add.at(pre1, row, val[:, None] * XW[col])
    h1 = _rrelu(pre1)
    M2 = np.abs(h1 @ W2f).max()
    k2 = int(np.floor(np.log2(16.0 / M2)))

    w1_eff = (W1f * 2.0**k1).astype(BF16)
    w2_eff = (W2f * 2.0 ** (k2 - k1)).astype(BF16)
    out_scale = 2.0**-k2

    return dict(
        CC=CC, segs=segs, NCH=NCH, SEGCH=SEGCH, base=base,
        idx=idx, sv=sv, xs=xs, w1=w1_eff, w2=w2_eff,
        ncore=ncore, nlocal=nlocal, out_scale=out_scale,
    )


def _build_program(CC, segs, NCH, SEGCH, sim1):
    import concourse.tile as tile
    from concourse import bacc, mybir
    from concourse.masks import make_identity
    from contextlib import ExitStack

    F32, F16, I16 = mybir.dt.float32, mybir.dt.float16, mybir.dt.int16
    BF = mybir.dt.bfloat16
    NSLOT = NCH * 128
    base = np.zeros(NWIN + 1, np.int64)
    base[1:] = np.cumsum(CC)

    nc = bacc.Bacc(
        "TRN2", target_bir_lowering=False, debug=False,
        num_devices=(1 if sim1 else NC),
    )
    xs_d = nc.dram_tensor("xs", [128, RTP], BF, kind="ExternalInput")
    w1_d = nc.dram_tensor("w1", [F, F], BF, kind="ExternalInput")
    w2_d = nc.dram_tensor("w2", [F, F], BF, kind="ExternalInput")
    idx_d = nc.dram_tensor("idx", [128, NSLOT // 16], I16, kind="ExternalInput")
    sv_d = nc.dram_tensor("sv", [128, NCH * 2 * W], F16, kind="ExternalInput")
    out_d = nc.dram_tensor("out", [RTP, F], F32, kind="ExternalOutput")

    with tile.TileContext(nc) as tc, ExitStack() as ctx:
        const = ctx.enter_context(tc.tile_pool(name="const", bufs=1))
        big = ctx.enter_context(tc.tile_pool(name="big", bufs=1))
        msgp = ctx.enter_context(tc.tile_pool(name="msgp", bufs=2))
        tps = ctx.enter_context(tc.tile_pool(name="tps", bufs=1, space="PSUM"))
        accp = ctx.enter_context(tc.tile_pool(name="accp", bufs=6, space="PSUM"))
        xtp = ctx.enter_context(tc.tile_pool(name="xtp", bufs=2))
        tsh = ctx.enter_context(tc.tile_pool(name="tsh", bufs=4))
        rrp = ctx.enter_context(tc.tile_pool(name="rrp", bufs=8))
        dram = ctx.enter_context(tc.tile_pool(name="dram", bufs=1, space="DRAM"))

        ident = const.tile([128, 128], BF)
        make_identity(nc, ident[:])
        w1_sb = const.tile([F, F], BF)
        nc.sync.dma_start(w1_sb[:], w1_d[:, :])
        w2_sb = const.tile([F, F], BF)
        nc.sync.dma_start(w2_sb[:], w2_d[:, :])
        xs_sb = big.tile([128, RTP], BF)
        nc.sync.dma_start(xs_sb[:], xs_d[:, :])
        idx_sb = big.tile([128, NSLOT // 16], I16)
        nc.sync.dma_start(idx_sb[:], idx_d[:, :])
        sv_sb = big.tile([128, NCH * 2 * W], F16)
        h1_sb = big.tile([128, NT * 128], BF)

        _aspace = "Local" if sim1 else "Shared"
        shard1 = dram.tile([RTP, F], F16)
        shard2 = dram.tile([RTP, F], F16)
        table1 = dram.tile([NC * RTP, F], F16, addr_space=_aspace)
        table2 = dram.tile([NC * RTP, F], F16, addr_space=_aspace)

        def build_tile(t, w_sb, shard, table, from_x):
            if from_x:
                lhsT = xs_sb[:, t * 128 : (t + 1) * 128]
            else:
                tp = tps.tile([128, 128], BF, tag="tp")
                nc.tensor.transpose(
                    tp[:], h1_sb[:, t * 128 : (t + 1) * 128], ident[:]
                )
                xt = xtp.tile([128, 128], BF, tag="xt")
                nc.scalar.activation(
                    xt[:], tp[:], mybir.ActivationFunctionType.Copy
                )
                lhsT = xt[:]
            mp = tps.tile([128, 128], F32, tag="mp")
            nc.tensor.matmul(
                out=mp[:], lhsT=lhsT, rhs=w_sb[:], start=True, stop=True
            )
            sh = tsh.tile([128, 128], F16, tag="sh")
            nc.scalar.activation(sh[:], mp[:], mybir.ActivationFunctionType.Copy)
            if sim1:
                for r in range(NC):
                    nc.sync.dma_start(
                        table[r * RTP + t * 128 : r * RTP + (t + 1) * 128, :],
                        sh[:],
                    )
            else:
                nc.sync.dma_start(shard[t * 128 : (t + 1) * 128, :], sh[:])

        def finish_table(shard, table):
            if not sim1:
                nc.gpsimd.collective_compute(
                    "AllGather",
                    mybir.AluOpType.bypass,
                    replica_groups=[list(range(NC))],
                    ins=[shard.opt()],
                    outs=[table.opt()],
                )

        def spmm(table, emit, load_sv, post_seg):
            tview = table[:, :].rearrange("(u two) f -> u (two f)", two=2)
            for w0, w1, c0, c1 in segs:
                nch = c1 - c0
                if load_sv:
                    nc.sync.dma_start(
                        sv_sb[:, c0 * 2 * W : c1 * 2 * W],
                        sv_d[:, c0 * 2 * W : c1 * 2 * W],
                    )
                msg = msgp.tile([128, SEGCH, 2 * F], F16, tag="msg")
                nc.gpsimd.dma_gather(
                    out_ap=msg[:, :nch, :],
                    in_ap=tview,
                    idxs_ap=idx_sb[:, c0 * 8 : c1 * 8],
                    num_idxs=nch * 128,
                    num_idxs_reg=nch * 128,
                    elem_size=2 * F,
                    single_packet=False,
                )
                for w in range(w0, w1):
                    acc = accp.tile([W, 128], F32, tag="acc")
                    ncw = int(CC[w])
                    b0 = int(base[w]) - c0
                    k = 0
                    for c in range(b0, b0 + ncw):
                        gc = c0 + c
                        for par in range(2):
                            nc.tensor.matmul(
                                out=acc[:],
                                lhsT=sv_sb[
                                    :,
                                    gc * 2 * W + par * W : gc * 2 * W
                                    + (par + 1) * W,
                                ],
                                rhs=msg[:, c, par * F : (par + 1) * F],
                                start=(k == 0),
                                stop=(k == 2 * ncw - 1),
                            )
                            k += 1
                    emit(w, acc)
                if post_seg is not None:
                    post_seg(w0, w1)

        # ---- layer 1 table
        for t in range(NT):
            build_tile(t, w1_sb, shard1, table1, from_x=True)
        finish_table(shard1, table1)

        def emit1(w, acc):
            tmp = rrp.tile([W, 128], F32, tag="t1")
            nc.scalar.activation(
                tmp[:], acc[:], mybir.ActivationFunctionType.Copy, scale=SLOPE
            )
            t = w // 4
            p0 = (w % 4) * W
            nc.vector.tensor_tensor(
                out=h1_sb[p0 : p0 + W, t * 128 : (t + 1) * 128],
                in0=tmp[:],
                in1=acc[:],
                op=mybir.AluOpType.max,
            )

        def post_seg1(w0, w1):
            for t in range(w0 // 4, w1 // 4):
                build_tile(t, w2_sb, shard2, table2, from_x=False)

        spmm(table1, emit1, load_sv=True, post_seg=post_seg1)
        finish_table(shard2, table2)

        def emit2(w, acc):
            tmp = rrp.tile([W, 128], F32, tag="t1")
            nc.scalar.activation(
                tmp[:], acc[:], mybir.ActivationFunctionType.Copy, scale=SLOPE
            )
            res = rrp.tile([W, 128], F32, tag="res")
            nc.vector.tensor_tensor(
                out=res[:], in0=tmp[:], in1=acc[:], op=mybir.AluOpType.max
            )
            nc.sync.dma_start(out_d[w * W : (w + 1) * W, :], res[:])

        spmm(table2, emit2, load_sv=False, post_seg=None)

    nc.compile()
    return nc


def kernel(
    features, adj_row, adj_col, adj_val,
    W1, g1_W, g1_U, g1_b, W2, g2_W, g2_U, g2_b,
    _run_kwargs=None,
):
    from concourse.bass_utils import run_bass_kernel_spmd

    prep = _host_prep(
        features, adj_row, adj_col, adj_val,
        W1, g1_W, g1_U, g1_b, W2, g2_W, g2_U, g2_b,
    )
    nc = _build_program(prep["CC"], prep["segs"], prep["NCH"], prep["SEGCH"], SIM1)

    in_maps = [
        {
            "xs": prep["xs"][i],
            "w1": prep["w1"],
            "w2": prep["w2"],
            "idx": prep["idx"][i],
            "sv": prep["sv"][i],
        }
        for i in range(NC)
    ]
    res = run_bass_kernel_spmd(
        nc, in_maps, core_ids=list(range(NC)), **(_run_kwargs or {})
    )
    ncore, nlocal = prep["ncore"], prep["nlocal"]
    s = prep["out_scale"]
    out = np.empty((N, F), np.float32)
    for i in range(NC):
        m = ncore == i
        out[m] = res.results[i]["out"][nlocal[m]] * s
    if _run_kwargs:
        kernel.last_results = res
    return out


# revision 17
# speedup vs baseline: 1.7592x; 1.7592x over previous
"""EvolveGCN (2-layer) Trainium2 Bass kernel, 8-way sharded. v2.

Algebraic reduction (same as v1): only h2[T-1] is returned and the mat-GRU
weight evolution is data-independent, so the whole model collapses to
    W1* = matGRU^4(W1);  W2* = matGRU^4(W2)      (host, fp64)
    h1  = rrelu(A3 @ (X3 @ W1*));  out = rrelu(A3 @ (h1 @ W2*))

v2 device scheme (per core, nodes row-partitioned):
- Node relabeling pi per core: 196 windows x 32 nodes, LPT-balanced by degree
  so every window receives ~510 edges -> the shared chunk schedule is
  ceil(max_core/128) = 4 chunks everywhere (~0.4% slot padding vs 16% in v1).
- Table (X@W1*, h1@W2*) is fp16 [50176, 128], built on device from bf16
  inputs, AllGathered (SIM1: emulated by 8 DMA copies). The gather views it
  as [25088, 256] super-rows: one 512B descriptor per edge (same modeled DMA
  cost as v1's 256B descriptor, but int16 super-row indices kill the A/B
  index-range split and its per-window double ceil).
- Messages: SWDGE dma_gather per segment (8 windows, ~32 chunks).
- Scatter: per chunk two one-hot fp16 matmuls (lo/hi half of each 512B slot)
  accumulating into a [32,128] PSUM window; S streamed once from DRAM
  (12.9MB) and kept resident for layer 2.
- rrelu emit split across ACT (x*SLOPE) and DVE (max) engines; h1 kept in
  SBUF as bf16; layer-2 table build (transpose + matmul) interleaved into
  the layer-1 segment loop. Output unscaling (2^-k2) done on host.
"""

import sys

for _p in ("/opt/trn_rl_repo",):
    if _p not in sys.path:
        sys.path.insert(0, _p)

import heapq

import ml_dtypes
import numpy as np

T, N, E, F = 4, 50000, 800000, 128
NC = 8
NPC = N // NC            # 6250 nodes per core
W = 32                   # window rows
NWIN = 196               # windows per core
RTP = NWIN * W           # 6272 padded rows per core
NT = RTP // 128          # 49 row tiles per core
SEG_WINS = 14            # windows per gather segment
SLOPE = 11.0 / 48.0      # torch RReLU eval negative slope

SIM1 = False             # single-core, no-collective variant for TimelineSim

BF16 = ml_dtypes.bfloat16


def _evolve(W0, gW, gU, gb, steps=T):
    def sig(x):
        return 1.0 / (1.0 + np.exp(-x))

    Q = W0.astype(np.float64)
    gW = gW.astype(np.float64)
    gU = gU.astype(np.float64)
    gb = gb.astype(np.float64)
    for _ in range(steps):
        z = sig(gW[0] @ Q + gU[0] @ Q + gb[0])
        r = sig(gW[1] @ Q + gU[1] @ Q + gb[1])
        h = np.tanh(gW[2] @ Q + gU[2] @ (r * Q) + gb[2])
        Q = (1.0 - z) * Q + z * h
    return Q.astype(np.float32)


def _lpt_windows(deg):
    """Assign all N nodes (by degree) to NC*NWIN global windows of W slots,
    balancing per-window degree sums. Nodes may land on any core — this
    balances core totals and window sums at once. Returns pos_g[node] in
    [0, NC*RTP)."""
    nbins = NC * NWIN
    order = np.argsort(-deg, kind="stable")
    pos_g = np.empty(N, np.int64)
    cnt = np.zeros(nbins, np.int32)
    heap = [(0.0, w) for w in range(nbins)]
    heapq.heapify(heap)
    for node in order:
        while True:
            s, w = heapq.heappop(heap)
            if cnt[w] < W:
                break
        pos_g[node] = w * W + cnt[w]
        cnt[w] += 1
        if cnt[w] < W:
            heapq.heappush(heap, (s + deg[node], w))
    return pos_g


def _rrelu(x):
    return np.where(x >= 0, x, SLOPE * x)


def _host_prep(features, adj_row, adj_col, adj_val, W1, g1_W, g1_U, g1_b,
               W2, g2_W, g2_U, g2_b):
    X = np.asarray(features[T - 1], dtype=np.float32)
    row = np.asarray(adj_row[T - 1], dtype=np.int64)
    col = np.asarray(adj_col[T - 1], dtype=np.int64)
    val = np.asarray(adj_val[T - 1], dtype=np.float32)

    W1f = _evolve(np.asarray(W1), np.asarray(g1_W), np.asarray(g1_U), np.asarray(g1_b))
    W2f = _evolve(np.asarray(W2), np.asarray(g2_W), np.asarray(g2_U), np.asarray(g2_b))

    # --- node relabeling: global LPT window balancing by (row-)degree;
    # a node's core is whichever window it lands in
    deg = np.bincount(row, minlength=N).astype(np.float64)
    newpos_g = _lpt_windows(deg)                                  # node -> table row

    trow_g = newpos_g[row]
    tcol_g = newpos_g[col]
    ecore = trow_g // RTP
    trl = trow_g % RTP
    ewin = trl // W
    erow = trl % W
    esup = tcol_g // 2
    epar = tcol_g % 2

    # --- shared chunk schedule
    counts = np.zeros((NC, NWIN), np.int64)
    np.add.at(counts, (ecore, ewin), 1)
    CC = np.maximum(1, -(-counts.max(axis=0) // 128))   # chunks per window
    base = np.zeros(NWIN + 1, np.int64)
    base[1:] = np.cumsum(CC)
    NCH = int(base[-1])
    NSLOT = NCH * 128

    segs = []
    for w0 in range(0, NWIN, SEG_WINS):
        w1 = min(w0 + SEG_WINS, NWIN)
        segs.append((w0, w1, int(base[w0]), int(base[w1])))
    SEGCH = max(c1 - c0 for _, _, c0, c1 in segs)

    # --- per-core slot data
    idx = np.zeros((NC, 128, NSLOT // 16), np.int16)
    sv = np.zeros((NC, 128, NCH * 2 * W), ml_dtypes.float8_e3m4)
    for i in range(NC):
        m = ecore == i
        w_, r_, s_, p_, v_ = ewin[m], erow[m], esup[m], epar[m], val[m]
        o = np.argsort(w_, kind="stable")
        w_, r_, s_, p_, v_ = w_[o], r_[o], s_[o], p_[o], v_[o]
        winstart = np.searchsorted(w_, np.arange(NWIN))
        pos = np.arange(w_.size) - winstart[w_]
        assert (pos < CC[w_] * 128).all()
        slot = base[w_] * 128 + pos
        flat = np.zeros(NSLOT, np.int16)
        flat[slot] = s_.astype(np.int16)
        wrap = flat.reshape(-1, 16).T
        idx[i] = np.tile(wrap, (8, 1))
        c_ = slot // 128
        pp_ = slot % 128
        sv[i][pp_, c_ * 2 * W + p_ * W + r_] = (v_ * 128.0).astype(ml_dtypes.float8_e3m4)

    # --- permuted, transposed, bf16 features
    ncore = newpos_g // RTP
    nlocal = newpos_g % RTP
    xs = np.zeros((NC, 128, RTP), BF16)
    for i in range(NC):
        m = ncore == i
        Xp = np.zeros((RTP, F), np.float32)
        Xp[nlocal[m]] = X[m]
        xs[i] = Xp.T.astype(BF16)

    # --- weight folding + pow2 scale calibration (keeps fp16 tables in a
    # comfortable range; inverse applied to the output on host)
    XW = X.astype(BF16).astype(np.float32) @ W1f
    k1 = int(np.floor(np.log2(16.0 / np.abs(XW).max())))
    try:
        from scipy.sparse import csr_matrix

        A = csr_matrix((val, (row, col)), shape=(N, N))
        pre1 = A @ XW
    except Exception:
        pre1 = np.zeros((N, F), np.float32)
        np.add.at(pre1, row, val[:, None] * XW[col])
    h1 = _rrelu(pre1)
    M2 = np.abs(h1 @ W2f).max()
    k2 = int(np.floor(np.log2(16.0 / M2)))

    w1_eff = (W1f * 2.0**k1).astype(BF16)
    w2_eff = (W2f * 2.0 ** (k2 - k1 - 7)).astype(BF16)
    out_scale = 2.0 ** -(k2 + 7)

    return dict(
        CC=CC, segs=segs, NCH=NCH, SEGCH=SEGCH, base=base,
        idx=idx, sv=sv, xs=xs, w1=w1_eff, w2=w2_eff,
        ncore=ncore, nlocal=nlocal, out_scale=out_scale,
    )


def _build_program(CC, segs, NCH, SEGCH, sim1, phase='all'):
    import concourse.tile as tile
    from concourse import bacc, mybir
    from concourse.masks import make_identity
    from contextlib import ExitStack

    F32, F16, I16 = mybir.dt.float32, mybir.dt.float16, mybir.dt.int16
    BF = mybir.dt.bfloat16
    NSLOT = NCH * 128
    base = np.zeros(NWIN + 1, np.int64)
    base[1:] = np.cumsum(CC)

    nc = bacc.Bacc(
        "TRN2", target_bir_lowering=False, debug=False,
        num_devices=(1 if sim1 else NC),
    )
    xs_d = nc.dram_tensor("xs", [128, RTP], BF, kind="ExternalInput")
    w1_d = nc.dram_tensor("w1", [F, F], BF, kind="ExternalInput")
    w2_d = nc.dram_tensor("w2", [F, F], BF, kind="ExternalInput")
    idx_d = nc.dram_tensor("idx", [128, NSLOT // 16], I16, kind="ExternalInput")
    sv_d = nc.dram_tensor("sv", [128, NCH * 2 * W], mybir.dt.float8e3, kind="ExternalInput")
    out_d = nc.dram_tensor("out", [RTP, F], F32, kind="ExternalOutput")

    with tile.TileContext(nc) as tc, ExitStack() as ctx:
        const = ctx.enter_context(tc.tile_pool(name="const", bufs=1))
        big = ctx.enter_context(tc.tile_pool(name="big", bufs=1))
        msgp = ctx.enter_context(tc.tile_pool(name="msgp", bufs=3))
        tps = ctx.enter_context(tc.tile_pool(name="tps", bufs=1, space="PSUM"))
        accp = ctx.enter_context(tc.tile_pool(name="accp", bufs=6, space="PSUM"))
        xtp = ctx.enter_context(tc.tile_pool(name="xtp", bufs=2))
        tsh = ctx.enter_context(tc.tile_pool(name="tsh", bufs=4))
        rrp = ctx.enter_context(tc.tile_pool(name="rrp", bufs=8))
        dram = ctx.enter_context(tc.tile_pool(name="dram", bufs=1, space="DRAM"))

        ident = const.tile([128, 128], BF)
        make_identity(nc, ident[:])
        w1_sb = const.tile([F, F], BF)
        nc.sync.dma_start(w1_sb[:], w1_d[:, :])
        w2_sb = const.tile([F, F], BF)
        nc.sync.dma_start(w2_sb[:], w2_d[:, :])
        xs_sb = big.tile([128, RTP], BF)
        nc.sync.dma_start(xs_sb[:], xs_d[:, :])
        idx_sb = big.tile([128, NSLOT // 16], I16)
        nc.sync.dma_start(idx_sb[:], idx_d[:, :])
        sv_sb = big.tile([128, NCH * 2 * W], mybir.dt.float8e3)
        h1_sb = big.tile([128, NT * 128], BF)

        _aspace = "Local" if sim1 else "Shared"
        shard1 = dram.tile([RTP, F], F16)
        shard2 = dram.tile([RTP, F], F16)
        table1 = dram.tile([NC * RTP, F], F16, addr_space=_aspace)
        table2 = dram.tile([NC * RTP, F], F16, addr_space=_aspace)

        def build_tile(t, w_sb, shard, table, from_x):
            if from_x:
                lhsT = xs_sb[:, t * 128 : (t + 1) * 128]
            else:
                tp = tps.tile([128, 128], BF, tag="tp")
                nc.tensor.transpose(
                    tp[:], h1_sb[:, t * 128 : (t + 1) * 128], ident[:]
                )
                xt = xtp.tile([128, 128], BF, tag="xt")
                nc.scalar.activation(
                    xt[:], tp[:], mybir.ActivationFunctionType.Copy
                )
                lhsT = xt[:]
            mp = tps.tile([128, 128], F32, tag="mp")
            nc.tensor.matmul(
                out=mp[:], lhsT=lhsT, rhs=w_sb[:], start=True, stop=True
            )
            sh = tsh.tile([128, 128], F16, tag="sh")
            nc.scalar.activation(sh[:], mp[:], mybir.ActivationFunctionType.Copy)
            nc.sync.dma_start(shard[t * 128 : (t + 1) * 128, :], sh[:])

        def copy_slice(shard, table, t0, t1):
            # SIM1 stand-in for the AllGather: replicate a finished shard
            # slice to all 8 table replicas (few, large DMA descriptors).
            for r in range(NC):
                nc.sync.dma_start(
                    table[r * RTP + t0 * 128 : r * RTP + t1 * 128, :],
                    shard[t0 * 128 : t1 * 128, :],
                )

        def finish_table(shard, table):
            if not sim1:
                nc.gpsimd.collective_compute(
                    "AllGather",
                    mybir.AluOpType.bypass,
                    replica_groups=[list(range(NC))],
                    ins=[shard.opt()],
                    outs=[table.opt()],
                )

        def spmm(table, emit, load_sv, post_seg):
            tview = table[:, :].rearrange("(u two) f -> u (two f)", two=2)
            for w0, w1, c0, c1 in segs:
                nch = c1 - c0
                if load_sv:
                    nc.sync.dma_start(
                        sv_sb[:, c0 * 2 * W : c1 * 2 * W],
                        sv_d[:, c0 * 2 * W : c1 * 2 * W],
                    )
                msg = msgp.tile([128, SEGCH, 2 * F], F16, tag="msg")
                if phase in ('l1_mm',):
                    nc.vector.memset(msg[:], 0)
                else:
                    nc.gpsimd.dma_gather(
                        out_ap=msg[:, :nch, :],
                        in_ap=tview,
                        idxs_ap=idx_sb[:, c0 * 8 : c1 * 8],
                        num_idxs=nch * 128,
                        num_idxs_reg=nch * 128,
                        elem_size=2 * F,
                        single_packet=False,
                    )
                if phase in ('l1_gather',):
                    continue
                for w in range(w0, w1):
                    acc = accp.tile([W, 128], F32, tag="acc")
                    ncw = int(CC[w])
                    b0 = int(base[w]) - c0
                    k = 0
                    for c in range(b0, b0 + ncw):
                        gc = c0 + c
                        for par in range(2):
                            nc.tensor.matmul(
                                out=acc[:],
                                lhsT=sv_sb[
                                    :,
                                    gc * 2 * W + par * W : gc * 2 * W
                                    + (par + 1) * W,
                                ],
                                rhs=msg[:, c, par * F : (par + 1) * F],
                                start=(k == 0),
                                stop=(k == 2 * ncw - 1),
                            )
                            k += 1
                    emit(w, acc)
                if post_seg is not None:
                    post_seg(w0, w1)

        # ---- layer 1 table
        SLICES = [(0, 13), (13, 26), (26, 39), (39, 46), (46, NT)]
        for t0, t1 in SLICES:
            for t in range(t0, t1):
                build_tile(t, w1_sb, shard1, table1, from_x=True)
            if sim1:
                copy_slice(shard1, table1, t0, t1)
        finish_table(shard1, table1)

        def emit1(w, acc):
            tmp = rrp.tile([W, 128], F32, tag="t1")
            nc.scalar.activation(
                tmp[:], acc[:], mybir.ActivationFunctionType.Copy, scale=SLOPE
            )
            t = w // 4
            p0 = (w % 4) * W
            nc.vector.tensor_tensor(
                out=h1_sb[p0 : p0 + W, t * 128 : (t + 1) * 128],
                in0=tmp[:],
                in1=acc[:],
                op=mybir.AluOpType.max,
            )

        t2_done = [0]

        def post_seg1(w0, w1):
            for t in range(w0 // 4, w1 // 4):
                build_tile(t, w2_sb, shard2, table2, from_x=False)
            prev = t2_done[0]
            t2_done[0] = w1 // 4
            if sim1:
                for t0, t1 in SLICES:
                    if prev < t1 <= t2_done[0]:
                        copy_slice(shard2, table2, t0, t1)

        _post1 = None if phase.startswith('l1') else post_seg1
        spmm(table1, emit1, load_sv=(phase not in ('l1_mm', 'l1_gather')), post_seg=_post1)
        if phase == 'all':
            finish_table(shard2, table2)

        ob = [None]

        def emit2(w, acc):
            tmp = rrp.tile([W, 128], F32, tag="t1")
            nc.scalar.activation(
                tmp[:], acc[:], mybir.ActivationFunctionType.Copy, scale=SLOPE
            )
            if w % 4 == 0:
                ob[0] = rrp.tile([128, 128], F32, tag="ob", name=f"ob_{w}")
            p0 = (w % 4) * W
            nc.vector.tensor_tensor(
                out=ob[0][p0 : p0 + W, :], in0=tmp[:], in1=acc[:],
                op=mybir.AluOpType.max,
            )
            if w % 4 == 3:
                t = w // 4
                nc.sync.dma_start(out_d[t * 128 : (t + 1) * 128, :], ob[0][:])

        if phase == 'all':
            spmm(table2, emit2, load_sv=False, post_seg=None)

    nc.compile()
    return nc


def kernel(
    features, adj_row, adj_col, adj_val,
    W1, g1_W, g1_U, g1_b, W2, g2_W, g2_U, g2_b,
    _run_kwargs=None,
):
    from concourse.bass_utils import run_bass_kernel_spmd

    prep = _host_prep(
        features, adj_row, adj_col, adj_val,
        W1, g1_W, g1_U, g1_b, W2, g2_W, g2_U, g2_b,
    )
    nc = _build_program(prep["CC"], prep["segs"], prep["NCH"], prep["SEGCH"], SIM1)

    in_maps = [
        {
            "xs": prep["xs"][i],
            "w1": prep["w1"],
            "w2": prep["w2"],
            "idx": prep["idx"][i],
            "sv": prep["sv"][i],
        }
        for i in range(NC)
    ]
    res = run_bass_kernel_spmd(
        nc, in_maps, core_ids=list(range(NC)), **(_run_kwargs or {})
    )
    ncore, nlocal = prep["ncore"], prep["nlocal"]
    s = prep["out_scale"]
    out = np.empty((N, F), np.float32)
    for i in range(NC):
        m = ncore == i
        out[m] = res.results[i]["out"][nlocal[m]] * s
    if _run_kwargs:
        kernel.last_results = res
    return out


# revision 23
# speedup vs baseline: 1.9495x; 1.1081x over previous
"""EvolveGCN (2-layer) Trainium2 Bass kernel, 8-way sharded. v2.

Algebraic reduction (same as v1): only h2[T-1] is returned and the mat-GRU
weight evolution is data-independent, so the whole model collapses to
    W1* = matGRU^4(W1);  W2* = matGRU^4(W2)      (host, fp64)
    h1  = rrelu(A3 @ (X3 @ W1*));  out = rrelu(A3 @ (h1 @ W2*))

v2 device scheme (per core, nodes row-partitioned):
- Node relabeling pi per core: 196 windows x 32 nodes, LPT-balanced by degree
  so every window receives ~510 edges -> the shared chunk schedule is
  ceil(max_core/128) = 4 chunks everywhere (~0.4% slot padding vs 16% in v1).
- Table (X@W1*, h1@W2*) is fp16 [50176, 128], built on device from bf16
  inputs, AllGathered (SIM1: emulated by 8 DMA copies). The gather views it
  as [25088, 256] super-rows: one 512B descriptor per edge (same modeled DMA
  cost as v1's 256B descriptor, but int16 super-row indices kill the A/B
  index-range split and its per-window double ceil).
- Messages: SWDGE dma_gather per segment (8 windows, ~32 chunks).
- Scatter: per chunk two one-hot fp16 matmuls (lo/hi half of each 512B slot)
  accumulating into a [32,128] PSUM window; S streamed once from DRAM
  (12.9MB) and kept resident for layer 2.
- rrelu emit split across ACT (x*SLOPE) and DVE (max) engines; h1 kept in
  SBUF as bf16; layer-2 table build (transpose + matmul) interleaved into
  the layer-1 segment loop. Output unscaling (2^-k2) done on host.
"""

import sys

for _p in ("/opt/trn_rl_repo",):
    if _p not in sys.path:
        sys.path.insert(0, _p)

import heapq

import ml_dtypes
import numpy as np

T, N, E, F = 4, 50000, 800000, 128
NC = 8
NPC = N // NC            # 6250 nodes per core
W = 32                   # window rows
NWIN = 196               # windows per core
RTP = NWIN * W           # 6272 padded rows per core
NT = RTP // 128          # 49 row tiles per core
SEG_WINS = 14            # windows per gather segment
SLOPE = 11.0 / 48.0      # torch RReLU eval negative slope

SIM1 = False             # single-core, no-collective variant for TimelineSim

BF16 = ml_dtypes.bfloat16


def _evolve(W0, gW, gU, gb, steps=T):
    def sig(x):
        return 1.0 / (1.0 + np.exp(-x))

    Q = W0.astype(np.float64)
    gW = gW.astype(np.float64)
    gU = gU.astype(np.float64)
    gb = gb.astype(np.float64)
    for _ in range(steps):
        z = sig(gW[0] @ Q + gU[0] @ Q + gb[0])
        r = sig(gW[1] @ Q + gU[1] @ Q + gb[1])
        h = np.tanh(gW[2] @ Q + gU[2] @ (r * Q) + gb[2])
        Q = (1.0 - z) * Q + z * h
    return Q.astype(np.float32)


def _lpt_windows(deg):
    """Assign all N nodes (by degree) to NC*NWIN global windows of W slots,
    balancing per-window degree sums. Nodes may land on any core — this
    balances core totals and window sums at once. Returns pos_g[node] in
    [0, NC*RTP)."""
    nbins = NC * NWIN
    order = np.argsort(-deg, kind="stable")
    pos_g = np.empty(N, np.int64)
    cnt = np.zeros(nbins, np.int32)
    heap = [(0.0, w) for w in range(nbins)]
    heapq.heapify(heap)
    for node in order:
        while True:
            s, w = heapq.heappop(heap)
            if cnt[w] < W:
                break
        pos_g[node] = w * W + cnt[w]
        cnt[w] += 1
        if cnt[w] < W:
            heapq.heappush(heap, (s + deg[node], w))
    return pos_g


def _rrelu(x):
    return np.where(x >= 0, x, SLOPE * x)


def _host_prep(features, adj_row, adj_col, adj_val, W1, g1_W, g1_U, g1_b,
               W2, g2_W, g2_U, g2_b):
    X = np.asarray(features[T - 1], dtype=np.float32)
    row = np.asarray(adj_row[T - 1], dtype=np.int64)
    col = np.asarray(adj_col[T - 1], dtype=np.int64)
    val = np.asarray(adj_val[T - 1], dtype=np.float32)

    W1f = _evolve(np.asarray(W1), np.asarray(g1_W), np.asarray(g1_U), np.asarray(g1_b))
    W2f = _evolve(np.asarray(W2), np.asarray(g2_W), np.asarray(g2_U), np.asarray(g2_b))

    # --- node relabeling: global LPT window balancing by (row-)degree;
    # a node's core is whichever window it lands in
    deg = np.bincount(row, minlength=N).astype(np.float64)
    newpos_g = _lpt_windows(deg)                                  # node -> table row

    trow_g = newpos_g[row]
    tcol_g = newpos_g[col]
    ecore = trow_g // RTP
    trl = trow_g % RTP
    ewin = trl // W
    erow = trl % W
    esup = tcol_g // 2
    epar = tcol_g % 2

    # --- shared chunk schedule
    counts = np.zeros((NC, NWIN), np.int64)
    np.add.at(counts, (ecore, ewin), 1)
    CC = np.maximum(1, -(-counts.max(axis=0) // 128))   # chunks per window
    base = np.zeros(NWIN + 1, np.int64)
    base[1:] = np.cumsum(CC)
    NCH = int(base[-1])
    NSLOT = NCH * 128

    segs = []
    for w0 in range(0, NWIN, SEG_WINS):
        w1 = min(w0 + SEG_WINS, NWIN)
        segs.append((w0, w1, int(base[w0]), int(base[w1])))
    SEGCH = max(c1 - c0 for _, _, c0, c1 in segs)

    # --- per-core slot data
    idx = np.zeros((NC, 128, NSLOT // 16), np.int16)
    sv = np.zeros((NC, 128, NCH * 2 * W), ml_dtypes.float8_e3m4)
    for i in range(NC):
        m = ecore == i
        w_, r_, s_, p_, v_ = ewin[m], erow[m], esup[m], epar[m], val[m]
        o = np.argsort(w_, kind="stable")
        w_, r_, s_, p_, v_ = w_[o], r_[o], s_[o], p_[o], v_[o]
        winstart = np.searchsorted(w_, np.arange(NWIN))
        pos = np.arange(w_.size) - winstart[w_]
        assert (pos < CC[w_] * 128).all()
        slot = base[w_] * 128 + pos
        flat = np.zeros(NSLOT, np.int16)
        flat[slot] = s_.astype(np.int16)
        wrap = flat.reshape(-1, 16).T
        idx[i] = np.tile(wrap, (8, 1))
        c_ = slot // 128
        pp_ = slot % 128
        sv[i][pp_, c_ * 2 * W + p_ * W + r_] = (v_ * 128.0).astype(ml_dtypes.float8_e3m4)

    # --- permuted, transposed, bf16 features
    ncore = newpos_g // RTP
    nlocal = newpos_g % RTP
    xs = np.zeros((NC, 128, RTP), BF16)
    for i in range(NC):
        m = ncore == i
        Xp = np.zeros((RTP, F), np.float32)
        Xp[nlocal[m]] = X[m]
        xs[i] = Xp.T.astype(BF16)

    # --- weight folding + pow2 scale calibration (keeps fp16 tables in a
    # comfortable range; inverse applied to the output on host)
    XW = X.astype(BF16).astype(np.float32) @ W1f
    k1 = int(np.floor(np.log2(16.0 / np.abs(XW).max())))
    try:
        from scipy.sparse import csr_matrix

        A = csr_matrix((val, (row, col)), shape=(N, N))
        pre1 = A @ XW
    except Exception:
        pre1 = np.zeros((N, F), np.float32)
        np.add.at(pre1, row, val[:, None] * XW[col])
    h1 = _rrelu(pre1)
    M2 = np.abs(h1 @ W2f).max()
    k2 = int(np.floor(np.log2(16.0 / M2)))

    w1_eff = (W1f * 2.0**k1).astype(BF16)
    w2_eff = (W2f * 2.0 ** (k2 - k1 - 7)).astype(BF16)
    out_scale = 2.0 ** -(k2 + 7)

    return dict(
        CC=CC, segs=segs, NCH=NCH, SEGCH=SEGCH, base=base,
        idx=idx, sv=sv, xs=xs, w1=w1_eff, w2=w2_eff,
        ncore=ncore, nlocal=nlocal, out_scale=out_scale,
    )


def _build_program(CC, segs, NCH, SEGCH, sim1, phase='all'):
    import concourse.tile as tile
    from concourse import bacc, mybir
    from concourse.masks import make_identity
    from contextlib import ExitStack

    F32, F16, I16 = mybir.dt.float32, mybir.dt.float16, mybir.dt.int16
    BF = mybir.dt.bfloat16
    NSLOT = NCH * 128
    base = np.zeros(NWIN + 1, np.int64)
    base[1:] = np.cumsum(CC)

    nc = bacc.Bacc(
        "TRN2", target_bir_lowering=False, debug=False,
        num_devices=(1 if sim1 else NC),
    )
    xs_d = nc.dram_tensor("xs", [128, RTP], BF, kind="ExternalInput")
    w1_d = nc.dram_tensor("w1", [F, F], BF, kind="ExternalInput")
    w2_d = nc.dram_tensor("w2", [F, F], BF, kind="ExternalInput")
    idx_d = nc.dram_tensor("idx", [128, NSLOT // 16], I16, kind="ExternalInput")
    sv_d = nc.dram_tensor("sv", [128, NCH * 2 * W], mybir.dt.float8e3, kind="ExternalInput")
    out_d = nc.dram_tensor("out", [RTP, F], F32, kind="ExternalOutput")

    with tile.TileContext(nc) as tc, ExitStack() as ctx:
        const = ctx.enter_context(tc.tile_pool(name="const", bufs=1))
        big = ctx.enter_context(tc.tile_pool(name="big", bufs=1))
        msgp = ctx.enter_context(tc.tile_pool(name="msgp", bufs=3))
        tpp = ctx.enter_context(tc.tile_pool(name="tpp", bufs=1, space="PSUM"))
        mpp = ctx.enter_context(tc.tile_pool(name="mpp", bufs=2, space="PSUM"))
        accp = ctx.enter_context(tc.tile_pool(name="accp", bufs=5, space="PSUM"))
        xtp = ctx.enter_context(tc.tile_pool(name="xtp", bufs=2))
        stgp = ctx.enter_context(tc.tile_pool(name="stgp", bufs=2))
        outp = ctx.enter_context(tc.tile_pool(name="outp", bufs=2))
        rrp = ctx.enter_context(tc.tile_pool(name="rrp", bufs=8))
        dram = ctx.enter_context(tc.tile_pool(name="dram", bufs=1, space="DRAM"))

        ident = const.tile([128, 128], BF)
        make_identity(nc, ident[:])
        w1_sb = const.tile([F, F], BF)
        nc.sync.dma_start(w1_sb[:], w1_d[:, :])
        w2_sb = const.tile([F, F], BF)
        nc.sync.dma_start(w2_sb[:], w2_d[:, :])
        xs_sb = big.tile([128, RTP], BF)
        idx_sb = big.tile([128, NSLOT // 16], I16)
        nc.sync.dma_start(idx_sb[:], idx_d[:, :])
        sv_sb = big.tile([128, NCH * 2 * W], mybir.dt.float8e3)
        h1_sb = big.tile([128, NT * 128], BF)

        _aspace = "Local" if sim1 else "Shared"
        shard1 = dram.tile([RTP, F], F16)
        shard2 = dram.tile([RTP, F], F16)
        table1 = dram.tile([NC * RTP, F], F16, addr_space=_aspace)
        table2 = dram.tile([NC * RTP, F], F16, addr_space=_aspace)

        def build_slice(t0, t1, w_sb, shard, table, from_x):
            nt = t1 - t0
            stag = stgp.tile([128, nt * 128], F16, tag="stg", name=f"stg_{t0}_{from_x}")
            if from_x:
                nc.sync.dma_start(
                    xs_sb[:, t0 * 128 : t1 * 128], xs_d[:, t0 * 128 : t1 * 128]
                )
            for t in range(t0, t1):
                if from_x:
                    lhsT = xs_sb[:, t * 128 : (t + 1) * 128]
                else:
                    tp = tpp.tile([128, 128], BF, tag="tp")
                    nc.tensor.transpose(
                        tp[:], h1_sb[:, t * 128 : (t + 1) * 128], ident[:]
                    )
                    xt = xtp.tile([128, 128], BF, tag="xt")
                    nc.scalar.activation(
                        xt[:], tp[:], mybir.ActivationFunctionType.Copy
                    )
                    lhsT = xt[:]
                mp = mpp.tile([128, 128], F32, tag="mp")
                nc.tensor.matmul(
                    out=mp[:], lhsT=lhsT, rhs=w_sb[:], start=True, stop=True
                )
                nc.scalar.activation(
                    stag[:, (t - t0) * 128 : (t - t0 + 1) * 128],
                    mp[:],
                    mybir.ActivationFunctionType.Copy,
                )
            nc.sync.dma_start(
                shard[t0 * 128 : t1 * 128, :].rearrange("(tt p) f -> p tt f", p=128),
                stag[:].rearrange("p (tt f) -> p tt f", f=128),
            )
            if sim1:
                copy_slice(shard, table, t0, t1)

        def copy_slice(shard, table, t0, t1):
            # SIM1 stand-in for the AllGather: replicate a finished shard
            # slice to all 8 table replicas (few, large DMA descriptors).
            for r in range(NC):
                nc.sync.dma_start(
                    table[r * RTP + t0 * 128 : r * RTP + t1 * 128, :],
                    shard[t0 * 128 : t1 * 128, :],
                )

        def finish_table(shard, table):
            if not sim1:
                nc.gpsimd.collective_compute(
                    "AllGather",
                    mybir.AluOpType.bypass,
                    replica_groups=[list(range(NC))],
                    ins=[shard.opt()],
                    outs=[table.opt()],
                )

        def spmm(table, emit, load_sv, post_seg, gather_only=False):
            tview = table[:, :].rearrange("(u two) f -> u (two f)", two=2)
            for w0, w1, c0, c1 in segs:
                nch = c1 - c0
                if load_sv:
                    nc.sync.dma_start(
                        sv_sb[:, c0 * 2 * W : c1 * 2 * W],
                        sv_d[:, c0 * 2 * W : c1 * 2 * W],
                    )
                msg = msgp.tile([128, SEGCH, 2 * F], F16, tag="msg")
                if phase in ('l1_mm',):
                    nc.vector.memset(msg[:], 0)
                else:
                    nc.gpsimd.dma_gather(
                        out_ap=msg[:, :nch, :],
                        in_ap=tview,
                        idxs_ap=idx_sb[:, c0 * 8 : c1 * 8],
                        num_idxs=nch * 128,
                        num_idxs_reg=nch * 128,
                        elem_size=2 * F,
                        single_packet=False,
                    )
                if gather_only:
                    continue
                for w in range(w0, w1):
                    acc = accp.tile([W, 128], F32, tag="acc")
                    ncw = int(CC[w])
                    b0 = int(base[w]) - c0
                    k = 0
                    for c in range(b0, b0 + ncw):
                        gc = c0 + c
                        for par in range(2):
                            nc.tensor.matmul(
                                out=acc[:],
                                lhsT=sv_sb[
                                    :,
                                    gc * 2 * W + par * W : gc * 2 * W
                                    + (par + 1) * W,
                                ],
                                rhs=msg[:, c, par * F : (par + 1) * F],
                                start=(k == 0),
                                stop=(k == 2 * ncw - 1),
                            )
                            k += 1
                    emit(w, acc)
                if post_seg is not None:
                    post_seg(w0, w1)

        # ---- layer 1 table
        SLICES = [(0, 13), (13, 26), (26, 39), (39, 46), (46, NT)]
        for t0, t1 in SLICES:
            build_slice(t0, t1, w1_sb, shard1, table1, from_x=True)
        finish_table(shard1, table1)

        def emit1(w, acc):
            tmp = rrp.tile([W, 128], F32, tag="t1")
            nc.scalar.activation(
                tmp[:], acc[:], mybir.ActivationFunctionType.Copy, scale=SLOPE
            )
            t = w // 4
            p0 = (w % 4) * W
            nc.vector.tensor_tensor(
                out=h1_sb[p0 : p0 + W, t * 128 : (t + 1) * 128],
                in0=tmp[:],
                in1=acc[:],
                op=mybir.AluOpType.max,
            )

        t2_done = [0]

        def post_seg1(w0, w1):
            prev = t2_done[0]
            t2_done[0] = w1 // 4
            for t0, t1 in SLICES:
                if prev < t1 <= t2_done[0]:
                    build_slice(t0, t1, w2_sb, shard2, table2, from_x=False)

        _post1 = None if phase.startswith('l1') else post_seg1
        spmm(table1, emit1, load_sv=(phase not in ('l1_mm', 'l1_gather')), post_seg=_post1, gather_only=(phase == 'l1_gather'))
        if phase in ('all', 'l2_gather'):
            finish_table(shard2, table2)

        OUT_SLICES = [(0, 13), (13, 26), (26, 39), (39, 47), (47, NT)]
        ob = [None, 0, 0]  # tile, t0, t1

        def emit2(w, acc):
            tmp = rrp.tile([W, 128], F32, tag="t1")
            nc.scalar.activation(
                tmp[:], acc[:], mybir.ActivationFunctionType.Copy, scale=SLOPE
            )
            t = w // 4
            if w % 4 == 0 and any(t == a for a, _ in OUT_SLICES):
                t0, t1 = next(x for x in OUT_SLICES if x[0] == t)
                ob[0] = outp.tile(
                    [128, (t1 - t0) * 128], F32, tag="ost", name=f"ost_{w}"
                )
                ob[1], ob[2] = t0, t1
            p0 = (w % 4) * W
            nc.vector.tensor_tensor(
                out=ob[0][p0 : p0 + W, (t - ob[1]) * 128 : (t - ob[1] + 1) * 128],
                in0=tmp[:],
                in1=acc[:],
                op=mybir.AluOpType.max,
            )
            if w == ob[2] * 4 - 1:
                nc.sync.dma_start(
                    out_d[ob[1] * 128 : ob[2] * 128, :].rearrange(
                        "(tt p) f -> p tt f", p=128
                    ),
                    ob[0][:].rearrange("p (tt f) -> p tt f", f=128),
                )

        if phase in ('all', 'l2_gather'):
            spmm(table2, emit2, load_sv=False, post_seg=None,
                 gather_only=(phase == 'l2_gather'))

    nc.compile()
    return nc


def kernel(
    features, adj_row, adj_col, adj_val,
    W1, g1_W, g1_U, g1_b, W2, g2_W, g2_U, g2_b,
    _run_kwargs=None,
):
    from concourse.bass_utils import run_bass_kernel_spmd

    prep = _host_prep(
        features, adj_row, adj_col, adj_val,
        W1, g1_W, g1_U, g1_b, W2, g2_W, g2_U, g2_b,
    )
    nc = _build_program(prep["CC"], prep["segs"], prep["NCH"], prep["SEGCH"], SIM1)

    in_maps = [
        {
            "xs": prep["xs"][i],
            "w1": prep["w1"],
            "w2": prep["w2"],
            "idx": prep["idx"][i],
            "sv": prep["sv"][i],
        }
        for i in range(NC)
    ]
    res = run_bass_kernel_spmd(
        nc, in_maps, core_ids=list(range(NC)), **(_run_kwargs or {})
    )
    ncore, nlocal = prep["ncore"], prep["nlocal"]
    s = prep["out_scale"]
    out = np.empty((N, F), np.float32)
    for i in range(NC):
        m = ncore == i
        out[m] = res.results[i]["out"][nlocal[m]] * s
    if _run_kwargs:
        kernel.last_results = res
    return out


# revision 26
# speedup vs baseline: 2.0662x; 1.0599x over previous
"""EvolveGCN (2-layer) Trainium2 Bass kernel, 8-way sharded. v2.

Algebraic reduction (same as v1): only h2[T-1] is returned and the mat-GRU
weight evolution is data-independent, so the whole model collapses to
    W1* = matGRU^4(W1);  W2* = matGRU^4(W2)      (host, fp64)
    h1  = rrelu(A3 @ (X3 @ W1*));  out = rrelu(A3 @ (h1 @ W2*))

v2 device scheme (per core, nodes row-partitioned):
- Node relabeling pi per core: 196 windows x 32 nodes, LPT-balanced by degree
  so every window receives ~510 edges -> the shared chunk schedule is
  ceil(max_core/128) = 4 chunks everywhere (~0.4% slot padding vs 16% in v1).
- Table (X@W1*, h1@W2*) is fp16 [50176, 128], built on device from bf16
  inputs, AllGathered (SIM1: emulated by 8 DMA copies). The gather views it
  as [25088, 256] super-rows: one 512B descriptor per edge (same modeled DMA
  cost as v1's 256B descriptor, but int16 super-row indices kill the A/B
  index-range split and its per-window double ceil).
- Messages: SWDGE dma_gather per segment (8 windows, ~32 chunks).
- Scatter: per chunk two one-hot fp16 matmuls (lo/hi half of each 512B slot)
  accumulating into a [32,128] PSUM window; S streamed once from DRAM
  (12.9MB) and kept resident for layer 2.
- rrelu emit split across ACT (x*SLOPE) and DVE (max) engines; h1 kept in
  SBUF as bf16; layer-2 table build (transpose + matmul) interleaved into
  the layer-1 segment loop. Output unscaling (2^-k2) done on host.
"""

import sys

for _p in ("/opt/trn_rl_repo",):
    if _p not in sys.path:
        sys.path.insert(0, _p)

import heapq

import ml_dtypes
import numpy as np

T, N, E, F = 4, 50000, 800000, 128
NC = 8
NPC = N // NC            # 6250 nodes per core
W = 32                   # window rows
NWIN = 196               # windows per core
RTP = NWIN * W           # 6272 padded rows per core
NT = RTP // 128          # 49 row tiles per core
SEG_WINS = 14            # windows per gather segment
SLOPE = 11.0 / 48.0      # torch RReLU eval negative slope

SIM1 = False             # single-core, no-collective variant for TimelineSim

BF16 = ml_dtypes.bfloat16


def _evolve(W0, gW, gU, gb, steps=T):
    def sig(x):
        return 1.0 / (1.0 + np.exp(-x))

    Q = W0.astype(np.float64)
    gW = gW.astype(np.float64)
    gU = gU.astype(np.float64)
    gb = gb.astype(np.float64)
    for _ in range(steps):
        z = sig(gW[0] @ Q + gU[0] @ Q + gb[0])
        r = sig(gW[1] @ Q + gU[1] @ Q + gb[1])
        h = np.tanh(gW[2] @ Q + gU[2] @ (r * Q) + gb[2])
        Q = (1.0 - z) * Q + z * h
    return Q.astype(np.float32)


def _lpt_windows(deg):
    """Assign all N nodes (by degree) to NC*NWIN global windows of W slots,
    balancing per-window degree sums. Nodes may land on any core — this
    balances core totals and window sums at once. Returns pos_g[node] in
    [0, NC*RTP)."""
    nbins = NC * NWIN
    order = np.argsort(-deg, kind="stable")
    pos_g = np.empty(N, np.int64)
    cnt = np.zeros(nbins, np.int32)
    heap = [(0.0, w) for w in range(nbins)]
    heapq.heapify(heap)
    for node in order:
        while True:
            s, w = heapq.heappop(heap)
            if cnt[w] < W:
                break
        pos_g[node] = w * W + cnt[w]
        cnt[w] += 1
        if cnt[w] < W:
            heapq.heappush(heap, (s + deg[node], w))
    return pos_g


def _rrelu(x):
    return np.where(x >= 0, x, SLOPE * x)


def _host_prep(features, adj_row, adj_col, adj_val, W1, g1_W, g1_U, g1_b,
               W2, g2_W, g2_U, g2_b):
    X = np.asarray(features[T - 1], dtype=np.float32)
    row = np.asarray(adj_row[T - 1], dtype=np.int64)
    col = np.asarray(adj_col[T - 1], dtype=np.int64)
    val = np.asarray(adj_val[T - 1], dtype=np.float32)

    W1f = _evolve(np.asarray(W1), np.asarray(g1_W), np.asarray(g1_U), np.asarray(g1_b))
    W2f = _evolve(np.asarray(W2), np.asarray(g2_W), np.asarray(g2_U), np.asarray(g2_b))

    # --- node relabeling: global LPT window balancing by (row-)degree;
    # a node's core is whichever window it lands in
    deg = np.bincount(row, minlength=N).astype(np.float64)
    newpos_g = _lpt_windows(deg)                                  # node -> table row

    trow_g = newpos_g[row]
    tcol_g = newpos_g[col]
    ecore = trow_g // RTP
    trl = trow_g % RTP
    ewin = trl // W
    erow = trl % W
    esup = tcol_g // 2
    epar = tcol_g % 2

    # --- shared chunk schedule
    counts = np.zeros((NC, NWIN), np.int64)
    np.add.at(counts, (ecore, ewin), 1)
    CC = np.maximum(1, -(-counts.max(axis=0) // 128))   # chunks per window
    base = np.zeros(NWIN + 1, np.int64)
    base[1:] = np.cumsum(CC)
    NCH = int(base[-1])
    NSLOT = NCH * 128

    seg_sizes = [SEG_WINS] * 12 + [12, 8, 4, 4]
    assert sum(seg_sizes) == NWIN
    segs = []
    w0 = 0
    for sz in seg_sizes:
        w1 = w0 + sz
        segs.append((w0, w1, int(base[w0]), int(base[w1])))
        w0 = w1
    SEGCH = max(c1 - c0 for _, _, c0, c1 in segs)

    # --- per-core slot data
    idx = np.zeros((NC, 128, NSLOT // 16), np.int16)
    sv = np.zeros((NC, 128, NCH * 2 * W), np.float16)
    for i in range(NC):
        m = ecore == i
        w_, r_, s_, p_, v_ = ewin[m], erow[m], esup[m], epar[m], val[m]
        o = np.argsort(w_, kind="stable")
        w_, r_, s_, p_, v_ = w_[o], r_[o], s_[o], p_[o], v_[o]
        winstart = np.searchsorted(w_, np.arange(NWIN))
        pos = np.arange(w_.size) - winstart[w_]
        assert (pos < CC[w_] * 128).all()
        slot = base[w_] * 128 + pos
        flat = np.zeros(NSLOT, np.int16)
        flat[slot] = s_.astype(np.int16)
        wrap = flat.reshape(-1, 16).T
        idx[i] = np.tile(wrap, (8, 1))
        c_ = slot // 128
        pp_ = slot % 128
        sv[i][pp_, c_ * 2 * W + p_ * W + r_] = v_.astype(np.float16)

    # --- permuted, transposed, bf16 features
    ncore = newpos_g // RTP
    nlocal = newpos_g % RTP
    xs = np.zeros((NC, 128, RTP), ml_dtypes.float8_e3m4)
    for i in range(NC):
        m = ncore == i
        Xp = np.zeros((RTP, F), np.float32)
        Xp[nlocal[m]] = X[m]
        xs[i] = Xp.T.astype(ml_dtypes.float8_e3m4)

    # --- weight folding + pow2 scale calibration (keeps fp16 tables in a
    # comfortable range; inverse applied to the output on host)
    XW = X.astype(BF16).astype(np.float32) @ W1f
    k1 = int(np.floor(np.log2(10.0 / np.abs(XW).max())))
    try:
        from scipy.sparse import csr_matrix

        A = csr_matrix((val, (row, col)), shape=(N, N))
        pre1 = A @ XW
    except Exception:
        pre1 = np.zeros((N, F), np.float32)
        np.add.at(pre1, row, val[:, None] * XW[col])
    h1 = _rrelu(pre1)
    M2 = np.abs(h1 @ W2f).max()
    k2 = int(np.floor(np.log2(10.0 / M2)))

    w1_eff = (W1f * 2.0**k1).astype(BF16)
    w2_eff = (W2f * 2.0 ** (k2 - k1)).astype(BF16)
    out_scale = 2.0**-k2

    return dict(
        CC=CC, segs=segs, NCH=NCH, SEGCH=SEGCH, base=base,
        idx=idx, sv=sv, xs=xs, w1=w1_eff, w2=w2_eff,
        ncore=ncore, nlocal=nlocal, out_scale=out_scale,
    )


def _build_program(CC, segs, NCH, SEGCH, sim1, phase='all'):
    import concourse.tile as tile
    from concourse import bacc, mybir
    from concourse.masks import make_identity
    from contextlib import ExitStack

    F32, F16, I16 = mybir.dt.float32, mybir.dt.float16, mybir.dt.int16
    BF = mybir.dt.bfloat16
    NSLOT = NCH * 128
    base = np.zeros(NWIN + 1, np.int64)
    base[1:] = np.cumsum(CC)

    nc = bacc.Bacc(
        "TRN2", target_bir_lowering=False, debug=False,
        num_devices=(1 if sim1 else NC),
    )
    xs_d = nc.dram_tensor("xs", [128, RTP], mybir.dt.float8e3, kind="ExternalInput")
    w1_d = nc.dram_tensor("w1", [F, F], BF, kind="ExternalInput")
    w2_d = nc.dram_tensor("w2", [F, F], BF, kind="ExternalInput")
    idx_d = nc.dram_tensor("idx", [128, NSLOT // 16], I16, kind="ExternalInput")
    sv_d = nc.dram_tensor("sv", [128, NCH * 2 * W], F16, kind="ExternalInput")
    out_d = nc.dram_tensor("out", [RTP, F], F32, kind="ExternalOutput")

    with tile.TileContext(nc) as tc, ExitStack() as ctx:
        const = ctx.enter_context(tc.tile_pool(name="const", bufs=1))
        big = ctx.enter_context(tc.tile_pool(name="big", bufs=1))
        msgp = ctx.enter_context(tc.tile_pool(name="msgp", bufs=3))
        tpp = ctx.enter_context(tc.tile_pool(name="tpp", bufs=1, space="PSUM"))
        mpp = ctx.enter_context(tc.tile_pool(name="mpp", bufs=2, space="PSUM"))
        accp = ctx.enter_context(tc.tile_pool(name="accp", bufs=5, space="PSUM"))
        xtp = ctx.enter_context(tc.tile_pool(name="xtp", bufs=2))
        stgp = ctx.enter_context(tc.tile_pool(name="stgp", bufs=2))
        outp = ctx.enter_context(tc.tile_pool(name="outp", bufs=2))
        rrp = ctx.enter_context(tc.tile_pool(name="rrp", bufs=8))
        dram = ctx.enter_context(tc.tile_pool(name="dram", bufs=1, space="DRAM"))

        ident = const.tile([128, 128], BF)
        make_identity(nc, ident[:])
        w1_sb = const.tile([F, F], BF)
        nc.sync.dma_start(w1_sb[:], w1_d[:, :])
        w2_sb = const.tile([F, F], BF)
        nc.sync.dma_start(w2_sb[:], w2_d[:, :])
        xs_sb = big.tile([128, RTP], mybir.dt.float8e3)
        idx_sb = big.tile([128, NSLOT // 16], I16)
        nc.sync.dma_start(idx_sb[:], idx_d[:, :])
        sv_sb = big.tile([128, NCH * 2 * W], F16)
        h1_sb = big.tile([128, NT * 128], BF)

        _aspace = "Local" if sim1 else "Shared"
        F8 = mybir.dt.float8e3
        shard1 = dram.tile([RTP, F], F8)
        shard2 = dram.tile([RTP, F], F8)
        table1 = dram.tile([NC * RTP, F], F8, addr_space=_aspace)
        table2 = dram.tile([NC * RTP, F], F8, addr_space=_aspace)

        def build_slice(t0, t1, w_sb, shard, table, from_x):
            nt = t1 - t0
            stag = stgp.tile([128, nt * 128], mybir.dt.float8e3, tag="stg", name=f"stg_{t0}_{from_x}")
            if from_x:
                nc.sync.dma_start(
                    xs_sb[:, t0 * 128 : t1 * 128], xs_d[:, t0 * 128 : t1 * 128]
                )
            for t in range(t0, t1):
                if from_x:
                    lhsT = xs_sb[:, t * 128 : (t + 1) * 128]
                else:
                    tp = tpp.tile([128, 128], BF, tag="tp")
                    nc.tensor.transpose(
                        tp[:], h1_sb[:, t * 128 : (t + 1) * 128], ident[:]
                    )
                    xt = xtp.tile([128, 128], BF, tag="xt")
                    nc.scalar.activation(
                        xt[:], tp[:], mybir.ActivationFunctionType.Copy
                    )
                    lhsT = xt[:]
                mp = mpp.tile([128, 128], F32, tag="mp")
                nc.tensor.matmul(
                    out=mp[:], lhsT=lhsT, rhs=w_sb[:], start=True, stop=True
                )
                nc.scalar.activation(
                    stag[:, (t - t0) * 128 : (t - t0 + 1) * 128],
                    mp[:],
                    mybir.ActivationFunctionType.Copy,
                )
            nc.sync.dma_start(
                shard[t0 * 128 : t1 * 128, :].rearrange("(tt p) f -> p tt f", p=128),
                stag[:].rearrange("p (tt f) -> p tt f", f=128),
            )
            if sim1:
                copy_slice(shard, table, t0, t1)

        def copy_slice(shard, table, t0, t1):
            # SIM1 stand-in for the AllGather: replicate a finished shard
            # slice to all 8 table replicas (few, large DMA descriptors).
            for r in range(NC):
                nc.sync.dma_start(
                    table[r * RTP + t0 * 128 : r * RTP + t1 * 128, :],
                    shard[t0 * 128 : t1 * 128, :],
                )

        def finish_table(shard, table):
            if not sim1:
                nc.gpsimd.collective_compute(
                    "AllGather",
                    mybir.AluOpType.bypass,
                    replica_groups=[list(range(NC))],
                    ins=[shard.opt()],
                    outs=[table.opt()],
                )

        def spmm(table, emit, load_sv, post_seg, gather_only=False):
            tview = table[:, :].rearrange("(u two) f -> u (two f)", two=2)
            for w0, w1, c0, c1 in segs:
                nch = c1 - c0
                if load_sv:
                    nc.sync.dma_start(
                        sv_sb[:, c0 * 2 * W : c1 * 2 * W],
                        sv_d[:, c0 * 2 * W : c1 * 2 * W],
                    )
                msg = msgp.tile([128, SEGCH, 2 * F], mybir.dt.float8e3, tag="msg")
                if phase in ('l1_mm',):
                    nc.vector.memset(msg[:], 0)
                else:
                    nc.gpsimd.dma_gather(
                        out_ap=msg[:, :nch, :],
                        in_ap=tview,
                        idxs_ap=idx_sb[:, c0 * 8 : c1 * 8],
                        num_idxs=nch * 128,
                        num_idxs_reg=nch * 128,
                        elem_size=2 * F,
                        single_packet=False,
                    )
                if gather_only:
                    continue
                for w in range(w0, w1):
                    acc = accp.tile([W, 128], F32, tag="acc")
                    ncw = int(CC[w])
                    b0 = int(base[w]) - c0
                    k = 0
                    for c in range(b0, b0 + ncw):
                        gc = c0 + c
                        for par in range(2):
                            nc.tensor.matmul(
                                out=acc[:],
                                lhsT=sv_sb[
                                    :,
                                    gc * 2 * W + par * W : gc * 2 * W
                                    + (par + 1) * W,
                                ],
                                rhs=msg[:, c, par * F : (par + 1) * F],
                                start=(k == 0),
                                stop=(k == 2 * ncw - 1),
                            )
                            k += 1
                    emit(w, acc)
                if post_seg is not None:
                    post_seg(w0, w1)

        # ---- layer 1 table
        SLICES1 = [(0, 13), (13, 26), (26, 39), (39, NT)]
        SLICES2 = [(0, 13), (13, 26), (26, 39), (39, 46), (46, 48), (48, NT)]
        for t0, t1 in SLICES1:
            build_slice(t0, t1, w1_sb, shard1, table1, from_x=True)
        finish_table(shard1, table1)

        def emit1(w, acc):
            tmp = rrp.tile([W, 128], F32, tag="t1")
            nc.scalar.activation(
                tmp[:], acc[:], mybir.ActivationFunctionType.Copy, scale=SLOPE
            )
            t = w // 4
            p0 = (w % 4) * W
            nc.vector.tensor_tensor(
                out=h1_sb[p0 : p0 + W, t * 128 : (t + 1) * 128],
                in0=tmp[:],
                in1=acc[:],
                op=mybir.AluOpType.max,
            )

        t2_done = [0]

        def post_seg1(w0, w1):
            prev = t2_done[0]
            t2_done[0] = w1 // 4
            for t0, t1 in SLICES2:
                if prev < t1 <= t2_done[0]:
                    build_slice(t0, t1, w2_sb, shard2, table2, from_x=False)

        _post1 = None if phase.startswith('l1') else post_seg1
        spmm(table1, emit1, load_sv=(phase not in ('l1_mm', 'l1_gather')), post_seg=_post1, gather_only=(phase == 'l1_gather'))
        if phase in ('all', 'l2_gather'):
            finish_table(shard2, table2)

        OUT_SLICES = [(0, 13), (13, 26), (26, 39), (39, 45), (45, 48), (48, NT)]
        ob = [None, 0, 0]  # tile, t0, t1

        def emit2(w, acc):
            tmp = rrp.tile([W, 128], F32, tag="t1")
            nc.scalar.activation(
                tmp[:], acc[:], mybir.ActivationFunctionType.Copy, scale=SLOPE
            )
            t = w // 4
            if w % 4 == 0 and any(t == a for a, _ in OUT_SLICES):
                t0, t1 = next(x for x in OUT_SLICES if x[0] == t)
                ob[0] = outp.tile(
                    [128, (t1 - t0) * 128], F32, tag="ost", name=f"ost_{w}"
                )
                ob[1], ob[2] = t0, t1
            p0 = (w % 4) * W
            nc.vector.tensor_tensor(
                out=ob[0][p0 : p0 + W, (t - ob[1]) * 128 : (t - ob[1] + 1) * 128],
                in0=tmp[:],
                in1=acc[:],
                op=mybir.AluOpType.max,
            )
            if w == ob[2] * 4 - 1:
                nc.sync.dma_start(
                    out_d[ob[1] * 128 : ob[2] * 128, :].rearrange(
                        "(tt p) f -> p tt f", p=128
                    ),
                    ob[0][:].rearrange("p (tt f) -> p tt f", f=128),
                )

        if phase in ('all', 'l2_gather'):
            spmm(table2, emit2, load_sv=False, post_seg=None,
                 gather_only=(phase == 'l2_gather'))

    nc.compile()
    return nc


def kernel(
    features, adj_row, adj_col, adj_val,
    W1, g1_W, g1_U, g1_b, W2, g2_W, g2_U, g2_b,
    _run_kwargs=None,
):
    from concourse.bass_utils import run_bass_kernel_spmd

    prep = _host_prep(
        features, adj_row, adj_col, adj_val,
        W1, g1_W, g1_U, g1_b, W2, g2_W, g2_U, g2_b,
    )
    nc = _build_program(prep["CC"], prep["segs"], prep["NCH"], prep["SEGCH"], SIM1)

    in_maps = [
        {
            "xs": prep["xs"][i],
            "w1": prep["w1"],
            "w2": prep["w2"],
            "idx": prep["idx"][i],
            "sv": prep["sv"][i],
        }
        for i in range(NC)
    ]
    res = run_bass_kernel_spmd(
        nc, in_maps, core_ids=list(range(NC)), **(_run_kwargs or {})
    )
    ncore, nlocal = prep["ncore"], prep["nlocal"]
    s = prep["out_scale"]
    out = np.empty((N, F), np.float32)
    for i in range(NC):
        m = ncore == i
        out[m] = res.results[i]["out"][nlocal[m]] * s
    if _run_kwargs:
        kernel.last_results = res
    return out
